# revision 14
# baseline (speedup 1.0000x reference)
"""GQA attention kernel for Trainium2, 8 NeuronCores.

Sharding: core c = (b, kv) with b = c//4 (batch), kv = c%4 (kv-head).
Each core computes its kv-head's K/V projections, the 4 query heads of
that group, their attention weights (written fully, f32), and a partial
o_proj contribution (summed over kv groups on the host).

Layout strategy (all matmul contractions live on SBUF partitions):
  - host passes x^T (feat-major) and pre-transposed weight slices
  - Q^T [256,2048], K^T [64,2048] computed d-major; V [2048,64] s-major
  - S  [q,s]  = (Q^T slice).T @ K^T   -> softmax along free dim -> attw
  - S^T [s,q] = (K^T slice).T @ Q^T   -> exp -> Pt; AV^T = V.T @ Pt
  - normalization 1/l applied per-partition on W, and on AV^T via a
    PE-transposed + DMA-broadcast reciprocal row vector
  - o_proj: out[q,o] = (A^T slice).T @ Wo^T slice, partial per core
Matmul inputs use float32r (single-pass PE fp32, ~1.0-1.5e-3 rel err).
"""

import os
import sys

sys.path.insert(0, "/opt/trn_rl_repo")

import numpy as np

import concourse.bass as bass
import concourse.mybir as mybir
import concourse.tile as tile
from concourse import bacc, bass_utils
from concourse.bass import ds, ts
from concourse.masks import make_identity

S = 2048          # sequence length (q and kv)
D = 1024          # d_model
HD = 64           # head dim
G = 4             # query heads per core (per kv head)
GD = G * HD       # 256
F = D // 128      # 8 feature chunks
QT = S // 128     # 16 q tiles
SCALE = HD ** -0.5

F32 = mybir.dt.float32
F32R = mybir.dt.float32r
EXP = mybir.ActivationFunctionType.Exp

_CACHE = {}


def _build():
    nc = bacc.Bacc(
        "TRN2",
        target_bir_lowering=False,
        debug=False,
        enable_asserts=False,
        num_devices=8,
    )
    xq = nc.dram_tensor("xqT", (D, S), F32R, kind="ExternalInput").ap()
    xk = nc.dram_tensor("xkT", (D, S), F32R, kind="ExternalInput").ap()
    xv = nc.dram_tensor("xvT", (D, S), F32R, kind="ExternalInput").ap()
    wq = nc.dram_tensor("wqT", (D, GD), F32R, kind="ExternalInput").ap()
    wk = nc.dram_tensor("wkT", (D, HD), F32R, kind="ExternalInput").ap()
    wv = nc.dram_tensor("wvT", (D, HD), F32R, kind="ExternalInput").ap()
    wo = nc.dram_tensor("woT", (GD, D), F32R, kind="ExternalInput").ap()
    attw = nc.dram_tensor("attw", (G, S, S), F32, kind="ExternalOutput").ap()
    outp = nc.dram_tensor("outp", (S, D), F32, kind="ExternalOutput").ap()
    rt_dram = nc.dram_tensor("rt_scratch", (G, QT, 128), F32, kind="Internal").ap()

    with tile.TileContext(nc) as tc:
        with (
            tc.tile_pool(name="singles", bufs=1) as singles,
            tc.tile_pool(name="px", bufs=8) as px,
            tc.tile_pool(name="pP", bufs=6) as pP,
            tc.tile_pool(name="pPt", bufs=4) as pPt,
            tc.tile_pool(name="pO", bufs=3) as pO,
            tc.tile_pool(name="psmall", bufs=4) as psmall,
            tc.tile_pool(name="prbc", bufs=3) as prbc,
            tc.tile_pool(name="ps_a", bufs=2, space="PSUM") as ps_a,
            tc.tile_pool(name="ps_b", bufs=2, space="PSUM") as ps_b,
            tc.tile_pool(name="ps_c", bufs=2, space="PSUM") as ps_c,
        ):
            ident = singles.tile([128, 128], F32)
            make_identity(nc, ident)

            wq_sb = singles.tile([128, F, GD], F32R)
            nc.sync.dma_start(out=wq_sb, in_=wq.rearrange("(c p) m -> p c m", p=128))
            wk_sb = singles.tile([128, F, HD], F32R)
            nc.sync.dma_start(out=wk_sb, in_=wk.rearrange("(c p) m -> p c m", p=128))
            wv_sb = singles.tile([128, F, HD], F32R)
            nc.sync.dma_start(out=wv_sb, in_=wv.rearrange("(c p) m -> p c m", p=128))
            wo_sb = singles.tile([128, 2, D], F32R)
            nc.sync.dma_start(out=wo_sb, in_=wo.rearrange("(g p) o -> p g o", p=128))

            QT_sb = singles.tile([128, 2, S], F32R)
            # K^T duplicated into both partition halves so score matmuls can
            # use the same base partition as the Q^T slice of any head
            KT_sb = singles.tile([128, S], F32R)
            V_sb = singles.tile([128, QT, HD], F32R)
            AT_sb = singles.tile([128, 2, S], F32R)
            r_all = singles.tile([128, G * QT], F32)

            # ---- K projection: K^T[64, s] ----
            xk_sb = [px.tile([128, S], F32R, tag="x", name=f"xk{c}") for c in range(F)]
            for c in range(F):
                nc.sync.dma_start(out=xk_sb[c], in_=xk[c * 128:(c + 1) * 128, :])
            for half in range(2):
                pk = ps_a.tile([64, 1024], F32, tag="pa")
                for c in range(F):
                    for sc in range(2):
                        nc.tensor.matmul(
                            pk[:, ds(sc * 512, 512)],
                            wk_sb[:, c, :],
                            xk_sb[c][:, ds(half * 1024 + sc * 512, 512)],
                            start=(c == 0),
                            stop=(c == F - 1),
                        )
                nc.vector.tensor_copy(
                    out=KT_sb[ds(0, 64), ds(half * 1024, 1024)], in_=pk
                )
                nc.vector.tensor_copy(
                    out=KT_sb[ds(64, 64), ds(half * 1024, 1024)], in_=pk
                )

            # ---- Q projection: Q^T[256, q] ----
            xq_sb = [px.tile([128, S], F32R, tag="x", name=f"xq{c}") for c in range(F)]
            for c in range(F):
                nc.sync.dma_start(out=xq_sb[c], in_=xq[c * 128:(c + 1) * 128, :])
            for dg in range(2):
                for half in range(2):
                    pq = ps_a.tile([128, 1024], F32, tag="pa")
                    for c in range(F):
                        for sc in range(2):
                            nc.tensor.matmul(
                                pq[:, ds(sc * 512, 512)],
                                wq_sb[:, c, ds(dg * 128, 128)],
                                xq_sb[c][:, ds(sc * 512 + half * 1024, 512)],
                                start=(c == 0),
                                stop=(c == F - 1),
                            )
                    nc.vector.tensor_copy(
                        out=QT_sb[:, dg, ds(half * 1024, 1024)], in_=pq
                    )

            # ---- V projection (emitted lazily, after head 0 phase A) ----
            def emit_v_proj():
                xv_sb = [
                    px.tile([128, S], F32R, tag="x", name=f"xv{c}")
                    for c in range(F)
                ]
                for c in range(F):
                    nc.sync.dma_start(
                        out=xv_sb[c], in_=xv[c * 128:(c + 1) * 128, :]
                    )
                for st in range(QT):
                    pv = ps_c.tile([128, HD], F32, tag="avt")
                    for c in range(F):
                        nc.tensor.matmul(
                            pv,
                            xv_sb[c][:, ts(st, 128)],
                            wv_sb[:, c, :],
                            start=(c == 0),
                            stop=(c == F - 1),
                        )
                    nc.vector.tensor_copy(out=V_sb[:, st, :], in_=pv)

            # ---- attention per head ----
            for h in range(G):
                p0 = 64 * (h % 2)
                hg = h // 2
                # phase A: S[q,s] -> exp -> l -> W -> HBM
                def emit_a_tile(qt, h=h, p0=p0, hg=hg):
                    P_sb = pP.tile([128, S], F32, tag="P")
                    for half in range(2):
                        pa = ps_a.tile([128, 1024], F32, tag="pa")
                        for sc in range(2):
                            nc.tensor.matmul(
                                pa[:, ds(sc * 512, 512)],
                                QT_sb[ds(p0, 64), hg, ts(qt, 128)],
                                KT_sb[ds(p0, 64), ds(half * 1024 + sc * 512, 512)],
                                start=True,
                                stop=True,
                            )
                        # no accum_out: accumulation roughly doubles the
                        # activation's engine time; row-sum on DVE instead
                        nc.scalar.activation(
                            out=P_sb[:, ds(half * 1024, 1024)],
                            in_=pa,
                            func=EXP,
                            scale=SCALE,
                        )
                    l = psmall.tile([128, 1], F32, tag="l")
                    nc.vector.reduce_sum(
                        out=l, in_=P_sb, axis=mybir.AxisListType.X
                    )
                    idx = h * QT + qt
                    nc.vector.reciprocal(out=r_all[:, ds(idx, 1)], in_=l)
                    # normalize on GpSimd: scalar-as-AP blocks the DVE 2x
                    # read-port mode, so DVE has no advantage here and
                    # GpSimd is otherwise idle
                    nc.gpsimd.tensor_scalar_mul(
                        out=P_sb, in0=P_sb, scalar1=r_all[:, ds(idx, 1)]
                    )
                    nc.sync.dma_start(out=attw[h, ts(qt, 128), :], in_=P_sb)

                # phase B: S^T -> exp -> AV^T accumulate -> normalize -> A^T
                # interleaved with phase A at q-block granularity: emit the
                # 4 A-tiles of block qb, transpose/bounce their r values,
                # then B of block qb
                for qb in range(4):
                    for qt in range(qb * 4, qb * 4 + 4):
                        emit_a_tile(qt)
                    if h == 0 and qb == 0:
                        emit_v_proj()
                    ptr = ps_b.tile([4, 128], F32, tag="st")
                    nc.tensor.transpose(
                        ptr, r_all[:, ds(h * QT + qb * 4, 4)], ident
                    )
                    rT_sb = psmall.tile([4, 128], F32, tag="rT")
                    nc.vector.tensor_copy(out=rT_sb, in_=ptr)
                    nc.sync.dma_start(
                        out=rt_dram[h, qb * 4:(qb + 1) * 4, :], in_=rT_sb
                    )
                    pavt = ps_c.tile([64, 512], F32, tag="avt")
                    for st in range(QT):
                        pst = ps_b.tile([128, 512], F32, tag="st")
                        nc.tensor.matmul(
                            pst,
                            KT_sb[ds(p0, 64), ts(st, 128)],
                            QT_sb[ds(p0, 64), hg, ds(qb * 512, 512)],
                            start=True,
                            stop=True,
                        )
                        Pt = pPt.tile([128, 512], F32R, tag="Pt")
                        nc.scalar.activation(out=Pt, in_=pst, func=EXP, scale=SCALE)
                        nc.tensor.matmul(
                            pavt,
                            V_sb[:, st, :],
                            Pt,
                            start=(st == 0),
                            stop=(st == QT - 1),
                        )
                    # broadcast rT rows for this (h, qb) to 64 partitions
                    rbc = prbc.tile([64, 4, 128], F32, tag="rbc")
                    rsrc = rt_dram[h, qb * 4:(qb + 1) * 4, :]
                    rsrc_bc = bass.AP(
                        tensor=rsrc.tensor,
                        offset=rsrc.offset,
                        ap=[[0, 64]] + list(rsrc.ap),
                    )
                    nc.gpsimd.dma_start(out=rbc, in_=rsrc_bc)
                    nc.vector.tensor_tensor(
                        out=AT_sb[ds(p0, 64), hg, ds(qb * 512, 512)],
                        in0=pavt,
                        in1=rbc,
                        op=mybir.AluOpType.mult,
                    )
                    if h == G - 1:
                        # all heads done for this q block: o_proj partial now
                        for qt in range(qb * 4, qb * 4 + 4):
                            O_sb = pO.tile([128, D], F32, tag="O")
                            for oc in range(2):
                                po = ps_b.tile([128, 512], F32, tag="st")
                                for g in range(2):
                                    nc.tensor.matmul(
                                        po,
                                        AT_sb[:, g, ts(qt, 128)],
                                        wo_sb[:, g, ds(oc * 512, 512)],
                                        start=(g == 0),
                                        stop=(g == 1),
                                    )
                                nc.vector.tensor_copy(
                                    out=O_sb[:, ds(oc * 512, 512)], in_=po
                                )
                            nc.sync.dma_start(out=outp[ts(qt, 128), :], in_=O_sb)

    nc.compile()
    return nc


def _get_nc():
    if "nc" not in _CACHE:
        _CACHE["nc"] = _build()
    return _CACHE["nc"]


def _reference_fallback(query, key, value, attn_mask, Wq, Wk, Wv, Wo):
    B, Sq, _ = query.shape
    Q = (query @ Wq.T).reshape(B, Sq, 16, HD)
    K = (key @ Wk.T).reshape(B, S, 4, HD)
    V = (value @ Wv.T).reshape(B, S, 4, HD)
    Qg = Q.reshape(B, Sq, 4, 4, HD)
    scores = np.einsum("bqkgd,bskd->bkgqs", Qg, K) * SCALE
    scores = scores + attn_mask[None, None, None, :, :]
    scores -= scores.max(axis=-1, keepdims=True)
    w = np.exp(scores)
    w /= w.sum(axis=-1, keepdims=True)
    out = np.einsum("bkgqs,bskd->bqkgd", w, V).reshape(B, Sq, 16 * HD)
    out = out @ Wo.T
    return out.astype(np.float32), w.reshape(B, 16, Sq, S).astype(np.float32)


def kernel(query, key, value, attn_mask, Wq, Wk, Wv, Wo):
    query = np.asarray(query, np.float32)
    key = np.asarray(key, np.float32)
    value = np.asarray(value, np.float32)
    attn_mask = np.asarray(attn_mask, np.float32)
    Wq = np.asarray(Wq, np.float32)
    Wk = np.asarray(Wk, np.float32)
    Wv = np.asarray(Wv, np.float32)
    Wo = np.asarray(Wo, np.float32)

    if attn_mask.shape != (S, S) or np.any(attn_mask != 0.0):
        return _reference_fallback(
            query, key, value, attn_mask, Wq, Wk, Wv, Wo
        )

    nc = _get_nc()
    qT = [np.ascontiguousarray(query[b].T) for b in range(2)]
    kT = [np.ascontiguousarray(key[b].T) for b in range(2)]
    vT = [np.ascontiguousarray(value[b].T) for b in range(2)]
    WqT = Wq.T
    WkT = Wk.T
    WvT = Wv.T
    WoT = Wo.T
    in_maps = []
    for c in range(8):
        b, kv = divmod(c, 4)
        in_maps.append({
            "xqT": qT[b],
            "xkT": kT[b],
            "xvT": vT[b],
            "wqT": np.ascontiguousarray(WqT[:, kv * GD:(kv + 1) * GD]),
            "wkT": np.ascontiguousarray(WkT[:, kv * HD:(kv + 1) * HD]),
            "wvT": np.ascontiguousarray(WvT[:, kv * HD:(kv + 1) * HD]),
            "woT": np.ascontiguousarray(WoT[kv * GD:(kv + 1) * GD, :]),
        })
    want_trace = bool(os.environ.get("KERNEL_TRACE"))
    if want_trace:
        try:
            from antenv.axon_hooks import get_axon_ntff_profile_hook  # noqa: F401
        except ImportError:
            want_trace = False
    os.environ["BASS_NEVER_TRACE"] = "1"
    res = bass_utils.run_bass_kernel_spmd(
        nc,
        in_maps,
        core_ids=list(range(8)),
        trace=want_trace,
    )
    _CACHE["last_result"] = res

    attn = np.empty((2, 16, S, S), np.float32)
    out = np.zeros((2, S, D), np.float32)
    for c in range(8):
        b, kv = divmod(c, 4)
        attn[b, kv * G:(kv + 1) * G] = res.results[c]["attw"]
        out[b] += res.results[c]["outp"]
    return out, attn
